# revision 2
# baseline (speedup 1.0000x reference)
"""Trainium2 Bass kernel for nn_Attention_44074954391673.

Sharding: 8 cores; core c -> batch b = c//4, heads [3*(c%4), 3*(c%4)+3).
All matmuls bf16 with fp32 PSUM accumulation.

Host-side scaling folds: Wq /= 8 (q k^T and q-side rel pre-scaled by
1/sqrt(dh)); k-side rel table E /= 8; ssan_w *= 8 (cancels q pre-scale).

Relative-position (Toeplitz) terms: windowed [128, 1152] matmuls against the
(reversed) distance-embedding table -> DRAM scratch -> diagonal-AP DMA
read-back, two q-tiles per DMA.  Q-side reads back skewed rows directly (r2);
K-side reads back skewed [r, l] tiles (g2) which the PE transposes into bf16
PSUM; rel sum r2+t2 is a single DVE add reading the bf16 PSUM transpose.

Scores PSUM group: packed-K64 QK^T + Id@rt + Id@(struct products).
Products (bias_i + absb_i) * struct_i split 7/3 across DVE/Pool stt ops;
one pair merged on Pool before the identity-matmul joins.
"""
import os
import sys

sys.path.insert(0, "/opt/trn_rl_repo")

from contextlib import ExitStack

import numpy as np
import ml_dtypes

import concourse.bass as bass
import concourse.mybir as mybir
import concourse.tile as tile
from concourse import bacc
from concourse.bass import ds
from concourse.bass_utils import run_bass_kernel_spmd

BF16 = mybir.dt.bfloat16
F32 = mybir.dt.float32
AF = mybir.ActivationFunctionType
OP = mybir.AluOpType

H, DH = 12, 64
B, L, D = 2, 1024, 768
NCHUNK = 6
WIN = 1152
NT = 8


def build_program(use_mask: bool, use_pbias: bool, reps: int = 1):
    nc = bacc.Bacc("TRN2", target_bir_lowering=False, debug=False, num_devices=8)

    def din(name, shape, dt=BF16):
        return nc.dram_tensor(name, shape, dt, kind="ExternalInput").ap()

    hsT = din("hsT", [128, NCHUNK, 1024])
    wq = din("wq", [128, NCHUNK, 192])
    wk = din("wk", [128, NCHUNK, 192])
    wv = din("wv", [128, NCHUNK, 192])
    ert = din("ert", [128, 2048])
    et = din("et", [128, 2048])
    ssw = din("ssw", [64, 5, 3, 64])
    struct = din("struct", [NT, 128, 5, 1024])
    absb = din("absb", [1, 16], F32)
    idb = din("idb", [128, 128])
    if use_mask:
        maskv = din("maskv", [1, 1024])
        onesv = din("onesv", [1, 128])
    if use_pbias:
        bqv = din("bqv", [1, 192])
        bkv = din("bkv", [1, 192])
        bvv = din("bvv", [1, 192])
        onesL = din("onesL", [1, 1024])
    out = nc.dram_tensor("out", [NT, 128, 192], F32, kind="ExternalOutput").ap()

    with tile.TileContext(nc) as tc:
        for _rep in range(reps):
          with ExitStack() as ctx:
            # ---------------- constants ----------------
            consts = ctx.enter_context(tc.tile_pool(name="consts", bufs=1))

            def cload(ap_in, shape=None, dt=BF16, name=None):
                t = consts.tile(shape, dt, name=name)
                nc.sync.dma_start(t, ap_in)
                return t

            sb_hsT = cload(hsT, name="hsT", shape=[128, NCHUNK, 1024])
            sb_wq = cload(wq, name="wq", shape=[128, NCHUNK, 192])
            sb_wk = cload(wk, name="wk", shape=[128, NCHUNK, 192])
            sb_wv = cload(wv, name="wv", shape=[128, NCHUNK, 192])
            sb_ert = cload(ert, name="ert", shape=[128, 2048])
            sb_et = cload(et, name="et", shape=[128, 2048])
            sb_ssw = cload(ssw, name="ssw", shape=[64, 5, 3, 64])
            sb_absb = cload(
                bass.AP(tensor=absb.tensor, offset=0, ap=[[0, 128], [1, 16]]),
                name="absb_sb", shape=[128, 16], dt=F32,
            )
            sb_idb = cload(idb, name="idb", shape=[128, 128])
            if use_mask:
                sb_mask = cload(maskv, name="maskv", shape=[1, 1024])
                sb_ones = cload(onesv, name="onesv", shape=[1, 128])
            if use_pbias:
                sb_bq = cload(bqv, name="bqv", shape=[1, 192])
                sb_bk = cload(bkv, name="bkv", shape=[1, 192])
                sb_bv = cload(bvv, name="bvv", shape=[1, 192])
                sb_onesL = cload(onesL, name="onesL", shape=[1, 1024])

            # struct tiles: fully resident, loaded once up-front (Act queue)
            sstp = ctx.enter_context(tc.tile_pool(name="sstp", bufs=1))
            st_tiles = []
            for t in range(NT):
                st = sstp.tile([128, 5, 1024], BF16, tag=f"st{t}", name=f"st{t}")
                nc.scalar.dma_start(st, struct[t])
                st_tiles.append(st)

            qkv = ctx.enter_context(tc.tile_pool(name="qkv", bufs=1))
            qd = [qkv.tile([128, 1024], BF16, tag=f"qd{h}", name=f"qd{h}") for h in range(3)]
            kd = [qkv.tile([128, 1024], BF16, tag=f"kd{h}", name=f"kd{h}") for h in range(3)]
            vsb = qkv.tile([128, NT, 192], BF16, name="vsb")

            # ---------------- projections (q/k emitted before win; v after) --
            pp = ctx.enter_context(tc.tile_pool(name="pp", bufs=2, space="PSUM"))
            ppb = ctx.enter_context(tc.tile_pool(name="ppb", bufs=2, space="PSUM"))
            ptmp = ctx.enter_context(tc.tile_pool(name="ptmp", bufs=1))
            vta = ptmp.tile([128, 1024], BF16, tag="vta", name="vta")
            vtb = ptmp.tile([64, 1024], BF16, tag="vtb", name="vtb")

            def proj_mms(w_sb, bias_sb, mlo, msz, n):
                ps = pp.tile([128, 512], F32, tag="proj", name="ps")
                for c in range(NCHUNK):
                    last = (c == NCHUNK - 1) and not use_pbias
                    nc.tensor.matmul(
                        ps[0:msz, :],
                        lhsT=w_sb[:, c, ds(mlo, msz)],
                        rhs=sb_hsT[:, c, ds(512 * n, 512)],
                        start=(c == 0), stop=last,
                    )
                if use_pbias:
                    nc.tensor.matmul(
                        ps[0:msz, :],
                        lhsT=bias_sb[0:1, ds(mlo, msz)],
                        rhs=sb_onesL[0:1, ds(512 * n, 512)],
                        start=False, stop=True,
                    )
                return ps

            def proj_qk():
                for w_sb, bias_sb, dup in (
                    (sb_wq, (sb_bq if use_pbias else None), qd),
                    (sb_wk, (sb_bk if use_pbias else None), kd),
                ):
                    for n in range(2):
                        sl = ds(512 * n, 512)
                        ps = proj_mms(w_sb, bias_sb, 0, 128, n)
                        nc.scalar.activation(dup[0][0:64, sl], ps[0:64, :], AF.Copy)
                        nc.scalar.activation(dup[1][64:128, sl], ps[64:128, :], AF.Copy)
                        ps = proj_mms(w_sb, bias_sb, 128, 64, n)
                        nc.scalar.activation(dup[2][0:64, sl], ps[0:64, :], AF.Copy)
                    for h, (src, dst) in enumerate(((0, 64), (64, 0), (0, 64))):
                        nc.sync.dma_start(
                            dup[h][dst:dst + 64, :], dup[h][src:src + 64, :]
                        )

            def proj_v():
                for n in range(2):
                    sl = ds(512 * n, 512)
                    ps = proj_mms(sb_wv, (sb_bv if use_pbias else None), 0, 128, n)
                    nc.scalar.activation(vta[:, sl], ps, AF.Copy)
                    ps = proj_mms(sb_wv, (sb_bv if use_pbias else None), 128, 64, n)
                    nc.scalar.activation(vtb[:, sl], ps[0:64, :], AF.Copy)
                for t in range(NT):
                    pst = ppb.tile([128, 128], BF16, tag="vtp", name="pst")
                    nc.tensor.matmul(
                        pst, lhsT=vta[:, ds(128 * t, 128)], rhs=sb_idb,
                        is_transpose=True, start=True, stop=True,
                    )
                    nc.scalar.activation(vsb[:, t, 0:128], pst, AF.Copy)
                    pst2 = ppb.tile([128, 64], BF16, tag="vtp2", name="pst2")
                    nc.tensor.matmul(
                        pst2, lhsT=vtb[:, ds(128 * t, 128)], rhs=sb_idb[0:64, 0:64],
                        is_transpose=True, start=True, stop=True,
                    )
                    nc.scalar.activation(vsb[:, t, 128:192], pst2, AF.Copy)

            dpool = ctx.enter_context(tc.tile_pool(name="dscr", bufs=1, space="DRAM"))

            # Unified PSUM pools: psS = 2-bank score tiles, putil = 1-bank
            # utility tiles (windowed rel chunks, bias, transposes, qw chunks,
            # ctx).  8 banks total.
            psS = ctx.enter_context(tc.tile_pool(name="psS", bufs=1, space="PSUM"))
            putil = ctx.enter_context(tc.tile_pool(name="putil", bufs=6, space="PSUM"))
            rg = ctx.enter_context(tc.tile_pool(name="rg", bufs=2))
            wev = ctx.enter_context(tc.tile_pool(name="wev", bufs=2))
            prp = ctx.enter_context(tc.tile_pool(name="prp", bufs=3))
            prob = ctx.enter_context(tc.tile_pool(name="prob", bufs=3))
            misc = ctx.enter_context(tc.tile_pool(name="misc", bufs=3))
            qwp = ctx.enter_context(tc.tile_pool(name="qwp", bufs=1))

            dramQs = [dpool.tile([NT, 128, WIN], BF16, tag=f"dq{h}", name=f"dq{h}")
                      for h in range(3)]
            dramKs = [dpool.tile([NT, 128, WIN], BF16, tag=f"dk{h}", name=f"dk{h}")
                      for h in range(3)]
            qw_sbs = [qwp.tile([128, 5, 1024], BF16, tag=f"qw{h}", name=f"qw{h}")
                      for h in range(3)]

            def win_qw(h):
                qT, kT = qd[h], kd[h]
                # B1/B2: windowed rel matmuls -> scratch (packed K=64 pairs);
                # one [128, 2, WIN] evac tile + one DMA per tile pair
                for src, rhs_tab, dst, alt in (
                    (qT, sb_ert, dramQs[h], 0), (kT, sb_et, dramKs[h], 1),
                ):
                    for t0 in range(0, NT, 2):
                        ev2 = wev.tile([128, 2, WIN], BF16, tag="wev", name="ev2")
                        for pi, tt in ((0, t0), (1, t0 + 1)):
                            win = 896 - 128 * tt
                            for ci, (c0, w) in enumerate(((0, 512), (512, 512), (1024, 128))):
                                pw = putil.tile([128, 512], F32, tag="u", name="pw")
                                nc.tensor.matmul(
                                    pw[:, 0:w],
                                    lhsT=src[64 * pi:64 * pi + 64, ds(128 * tt, 128)],
                                    rhs=rhs_tab[64 * pi:64 * pi + 64, ds(win + c0, w)],
                                    start=True, stop=True, tile_position=(64 * pi, 0),
                                )
                                if (ci + pi + alt) % 2 == 0:
                                    nc.scalar.activation(ev2[:, pi, ds(c0, w)], pw[:, 0:w], AF.Copy)
                                else:
                                    nc.vector.tensor_copy(ev2[:, pi, ds(c0, w)], pw[:, 0:w])
                        nc.vector.dma_start(
                            bass.AP(
                                tensor=dst.tensor,
                                offset=dst.offset + t0 * 128 * WIN,
                                ap=[[WIN, 128], [128 * WIN, 2], [1, WIN]],
                            ),
                            ev2,
                        )

                # B3: qw_i (chunked into 1-bank psum tiles) + dup
                qw_sb = qw_sbs[h]
                for i in range(5):
                    for n2 in range(2):
                        pq = putil.tile([64, 512], F32, tag="u", name="pq")
                        nc.tensor.matmul(
                            pq,
                            lhsT=sb_ssw[:, i, h, :],
                            rhs=qT[0:64, ds(512 * n2, 512)],
                            start=True, stop=True,
                        )
                        nc.scalar.activation(qw_sb[0:64, i, ds(512 * n2, 512)], pq, AF.Copy)
                for i in range(5):
                    nc.sync.dma_start(qw_sb[64:128, i, :], qw_sb[0:64, i, :])

            proj_qk()
            # window/qw phases for all heads ahead of B4; v projection after
            # (v is first needed in stageD of head 0, well into B4)
            for h in range(3):
                win_qw(h)
            proj_v()

            # product (i, half) -> engine: 7 on DVE, 3 on Pool; the (0,+1)
            # pair is merged on Pool before the joins
            POOL_SET = {(3, 1), (4, 0), (4, 1)}

            for h in range(3):
                qT, kT = qd[h], kd[h]
                dramQ, dramK, qw_sb = dramQs[h], dramKs[h], qw_sbs[h]

                state = {}

                def stageA_dma(t0):
                    # paired diag readbacks for tiles (t0, t0+1)
                    r2 = rg.tile([128, 2, 1024], BF16, tag="r2", name="r2")
                    nc.sync.dma_start(
                        r2,
                        bass.AP(
                            tensor=dramQ.tensor,
                            offset=dramQ.offset + t0 * 128 * WIN + 127,
                            ap=[[WIN - 1, 128], [128 * WIN, 2], [1, 1024]],
                        ),
                    )
                    g2 = rg.tile([128, NT, 256], BF16, tag="g2", name="g2")
                    nc.sync.dma_start(
                        g2,
                        bass.AP(
                            tensor=dramK.tensor,
                            offset=dramK.offset + 128 * t0 + 127,
                            ap=[[WIN - 1, 128], [128 * WIN, NT], [1, 256]],
                        ),
                    )
                    state[t0] = {"r2": r2, "g2": g2}
                    state[t0 + 1] = state[t0]

                def stageA(t):
                    d = state[t]
                    ii = t % 2
                    g2 = d["g2"]
                    pt2 = putil.tile([128, 1024], BF16, tag="u", name="pt2")
                    for j in range(NT):
                        nc.tensor.matmul(
                            pt2[:, ds(128 * j, 128)],
                            lhsT=g2[:, j, ds(128 * ii, 128)], rhs=sb_idb,
                            is_transpose=True,
                            start=(j == 0), stop=(j == NT - 1),
                        )
                    psc = psS.tile([128, 1024], F32, tag="s", name="psc")
                    nc.tensor.matmul(
                        psc[:, 0:512],
                        lhsT=qT[0:64, ds(128 * t, 128)], rhs=kT[0:64, 0:512],
                        start=True, stop=False, tile_position=(0, 0),
                    )
                    nc.tensor.matmul(
                        psc[:, 512:1024],
                        lhsT=qT[64:128, ds(128 * t, 128)],
                        rhs=kT[64:128, 512:1024],
                        start=True, stop=False, tile_position=(64, 0),
                    )
                    if use_mask:
                        for half in range(2):
                            sl = ds(512 * half, 512)
                            nc.tensor.matmul(
                                psc[:, sl], lhsT=sb_ones[0:1, :],
                                rhs=sb_mask[0:1, sl], start=False, stop=False,
                            )
                    d[("pt2", t)] = pt2
                    d[("psc", t)] = psc

                def biasmm(t, i, half):
                    sl = ds(512 * half, 512)
                    pb = putil.tile([128, 512], F32, tag="u", name="pb")
                    rr = 0 if (i % 2 == 0) else 64
                    nc.tensor.matmul(
                        pb,
                        lhsT=qw_sb[rr:rr + 64, i, ds(128 * t, 128)],
                        rhs=kT[rr:rr + 64, sl],
                        start=True, stop=True, tile_position=(rr, 0),
                    )
                    return pb

                def stageB(t):
                    d = state[t]
                    st = st_tiles[t]
                    ii = t % 2
                    # rel sum: DVE add of (q-side skewed rows) + (bf16 PSUM
                    # transpose of k-side) -- 2x mode, no Act evac needed
                    rt = prp.tile([128, 1024], BF16, tag="rt", name="rt")
                    nc.vector.tensor_tensor(
                        rt, d[("pt2", t)], d["r2"][:, ii, :], OP.add)
                    d[("rt", t)] = rt
                    sums = {}
                    for i in range(5):
                        for half in range(2):
                            pb = biasmm(t, i, half)
                            pr = prp.tile([128, 512], BF16, tag=f"pd{i}{half}", name="pr")
                            eng = nc.gpsimd if (i, half) in POOL_SET else nc.vector
                            eng.scalar_tensor_tensor(
                                pr, in0=pb,
                                scalar=sb_absb[:, ds(3 * i + h, 1)],
                                in1=st[:, i, ds(512 * half, 512)],
                                op0=OP.add, op1=OP.mult,
                            )
                            sums[(i, half)] = pr
                    # merge products 0+1 per half on Pool
                    for half in range(2):
                        sp = prp.tile([128, 512], BF16, tag=f"sp{half}", name="sp")
                        nc.gpsimd.tensor_tensor(
                            sp, sums.pop((0, half)), sums.pop((1, half)), OP.add)
                        sums[("sp", half)] = sp
                    d[("sums", t)] = sums

                def stageC(t):
                    d = state[t]
                    psc, rt, sums = d[("psc", t)], d[("rt", t)], d[("sums", t)]
                    for half in range(2):
                        sl = ds(512 * half, 512)
                        joins = [rt[:, sl]]
                        joins += [v for (kk, hh), v in sums.items() if hh == half]
                        for ji, j in enumerate(joins):
                            nc.tensor.matmul(
                                psc[:, sl], lhsT=sb_idb, rhs=j,
                                start=False,
                                stop=(half == 1 and ji == len(joins) - 1),
                            )
                    probs = prob.tile([128, 1024], BF16, tag="p", name="probs")
                    rsum = misc.tile([128, 1], F32, tag="rs", name="rsum")
                    nc.scalar.activation(probs, psc, AF.Exp, accum_out=rsum)
                    d[("probs", t)] = probs
                    d[("rsum", t)] = rsum

                def stageD(t):
                    d = state[t]
                    probs, rsum = d[("probs", t)], d[("rsum", t)]
                    ptps = putil.tile([128, 1024], BF16, tag="u", name="ptps")
                    for j in range(NT):
                        nc.tensor.matmul(
                            ptps[:, ds(128 * j, 128)],
                            lhsT=probs[:, ds(128 * j, 128)], rhs=sb_idb,
                            is_transpose=True,
                            start=(j == 0), stop=(j == NT - 1),
                        )
                    ptsb = misc.tile([128, 1024], BF16, tag="ptsb", name="ptsb")
                    nc.scalar.activation(ptsb, ptps, AF.Copy)
                    ctxps = putil.tile([128, 64], F32, tag="u", name="ctxps")
                    for j in range(NT):
                        nc.tensor.matmul(
                            ctxps,
                            lhsT=ptsb[:, ds(128 * j, 128)],
                            rhs=vsb[:, j, ds(64 * h, 64)],
                            start=(j == 0), stop=(j == NT - 1),
                        )
                    rec = misc.tile([128, 1], F32, tag="rc", name="rec")
                    nc.vector.reciprocal(rec, rsum)
                    cn = misc.tile([128, 64], F32, tag="cn", name="cn")
                    nc.vector.tensor_scalar_mul(cn, ctxps, rec)
                    nc.sync.dma_start(out[t, :, ds(64 * h, 64)], cn)

                # pipelined emission; pair-granular prefetch DMAs lead
                stageA_dma(0)
                stageA_dma(2)
                for k in range(NT + 2):
                    if k % 2 == 0 and k + 4 < NT + 2:
                        stageA_dma(k + 4)
                    if 0 <= k - 2:
                        stageD(k - 2)
                    if k < NT:
                        stageA(k)
                        stageB(k)
                    if 0 <= k - 1 < NT:
                        stageC(k - 1)

    nc.compile()
    return nc, out


_PROGRAM_CACHE = {}


def kernel(**inputs):
    hs = np.asarray(inputs["hidden_states"], np.float32)
    mask = np.asarray(inputs["attention_mask"], np.float32)
    struct = np.asarray(inputs["struct_matrix"], np.float32)
    Wq = np.asarray(inputs["Wq"], np.float32)
    bq = np.asarray(inputs["bq"], np.float32)
    Wk = np.asarray(inputs["Wk"], np.float32)
    bk = np.asarray(inputs["bk"], np.float32)
    Wv = np.asarray(inputs["Wv"], np.float32)
    bv = np.asarray(inputs["bv"], np.float32)
    E = np.asarray(inputs["dist_emb"], np.float32)
    ssw = np.asarray(inputs["ssan_w"], np.float32)
    absb = np.asarray(inputs["abs_bias"], np.float32)

    bf = ml_dtypes.bfloat16
    use_mask = bool(np.any(mask))
    use_pbias = bool(np.any(bq) or np.any(bk) or np.any(bv))

    key = (use_mask, use_pbias)
    if key not in _PROGRAM_CACHE:
        _PROGRAM_CACHE[key] = build_program(use_mask, use_pbias)
    nc, _ = _PROGRAM_CACHE[key]

    Epad = np.concatenate([E, np.zeros((1, DH), np.float32)])
    Erev = np.concatenate([E[::-1], np.zeros((1, DH), np.float32)])
    ert_half = np.ascontiguousarray(Erev.T)
    et_half = np.ascontiguousarray(Epad.T) / 8.0
    ert_np = np.concatenate([ert_half, ert_half], 0).astype(bf)
    et_np = np.concatenate([et_half, et_half], 0).astype(bf)
    idb_np = np.eye(128, dtype=np.float32).astype(bf)

    in_maps = []
    for c in range(8):
        b = c // 4
        h0 = 3 * (c % 4)
        hsT = hs[b].T
        m = {
            "hsT": np.ascontiguousarray(
                hsT.reshape(NCHUNK, 128, 1024).transpose(1, 0, 2)
            ).astype(bf),
            "wq": np.ascontiguousarray(
                (Wq[:, h0 * 64:(h0 + 3) * 64] / 8.0)
                .reshape(NCHUNK, 128, 192).transpose(1, 0, 2)
            ).astype(bf),
            "wk": np.ascontiguousarray(
                Wk[:, h0 * 64:(h0 + 3) * 64]
                .reshape(NCHUNK, 128, 192).transpose(1, 0, 2)
            ).astype(bf),
            "wv": np.ascontiguousarray(
                Wv[:, h0 * 64:(h0 + 3) * 64]
                .reshape(NCHUNK, 128, 192).transpose(1, 0, 2)
            ).astype(bf),
            "ert": ert_np,
            "et": et_np,
            "ssw": np.ascontiguousarray(
                (ssw[:, h0:h0 + 3] * 8.0).transpose(2, 0, 1, 3)
            ).astype(bf),
            "struct": np.ascontiguousarray(
                struct[:, b, 0].reshape(5, NT, 128, 1024).transpose(1, 2, 0, 3)
            ).astype(bf),
            "absb": np.concatenate(
                [absb[:, h0:h0 + 3].reshape(1, 15),
                 np.zeros((1, 1), np.float32)], 1
            ),
            "idb": idb_np,
        }
        if use_mask:
            m["maskv"] = mask[b, 0, 0].reshape(1, 1024).astype(bf)
            m["onesv"] = np.ones((1, 128), np.float32).astype(bf)
        if use_pbias:
            m["bqv"] = (bq[h0 * 64:(h0 + 3) * 64] / 8.0).reshape(1, 192).astype(bf)
            m["bkv"] = bk[h0 * 64:(h0 + 3) * 64].reshape(1, 192).astype(bf)
            m["bvv"] = bv[h0 * 64:(h0 + 3) * 64].reshape(1, 192).astype(bf)
            m["onesL"] = np.ones((1, 1024), np.float32).astype(bf)
        in_maps.append(m)

    res = run_bass_kernel_spmd(nc, in_maps, core_ids=list(range(8)))
    outs = [r["out"] for r in res.results]

    full = np.zeros((B, L, D), np.float32)
    for c in range(8):
        b = c // 4
        h0 = 3 * (c % 4)
        o = np.asarray(outs[c], np.float32).reshape(L, 192)
        for j in range(3):
            full[b, :, (h0 + j) * 64:(h0 + j + 1) * 64] = o[:, j * 64:(j + 1) * 64]
    return full


# revision 45
# speedup vs baseline: 1.0465x; 1.0465x over previous
"""Trainium2 Bass kernel for nn_Attention_44074954391673.

Sharding: 8 cores; core c -> batch b = c//4, heads [3*(c%4), 3*(c%4)+3).
All matmuls bf16 with fp32 PSUM accumulation.

Host-side scaling folds: Wq /= 8 (q k^T and q-side rel pre-scaled by
1/sqrt(dh)); k-side rel table E /= 8; ssan_w *= 8 (cancels q pre-scale).

Relative-position (Toeplitz) terms: windowed [128, 1152] matmuls against the
(reversed) distance-embedding table -> DRAM scratch -> diagonal-AP DMA
read-back, two q-tiles per DMA.  Q-side reads back skewed rows directly (r2);
K-side reads back skewed [r, l] tiles (g2) which the PE transposes into bf16
PSUM; rel sum r2+t2 is a single DVE add reading the bf16 PSUM transpose
directly (2x mode, no Act evacuation).

Scores PSUM group: packed-K64 QK^T + Id@rt + Id@(struct products).
Products (bias_i + absb_i) * struct_i: i0-i2 as DVE scalar_tensor_tensor
reading f32 PSUM; i3-i4 as Act evac (+absb) then in-place Pool multiply
(GPSIMD cannot access PSUM); products 0+1 merged on DVE (bf16 2x) before
the identity-matmul joins.

Schedule: per-head emission pipeline tuned so B4(h0) starts ~30us in
(k-side windows first, qw before windows, struct tiles resident and
loaded on demand, paired r2/g2 prefetches, per-phase PSUM pools so the
window phases of later heads overlap the main loop); the bias+product
stage runs one round ahead of the joins; exp split in halves to pipeline
with the joins.  TimelineSim: ~310us/core (baseline ~319us); measured
~307-435us/rep on HW (reps-difference, noisy).
"""
import os
import sys

sys.path.insert(0, "/opt/trn_rl_repo")

from contextlib import ExitStack

import numpy as np
import ml_dtypes

import concourse.bass as bass
import concourse.mybir as mybir
import concourse.tile as tile
from concourse import bacc
from concourse.bass import ds
from concourse.bass_utils import run_bass_kernel_spmd

BF16 = mybir.dt.bfloat16
F32 = mybir.dt.float32
AF = mybir.ActivationFunctionType
OP = mybir.AluOpType

H, DH = 12, 64
B, L, D = 2, 1024, 768
NCHUNK = 6
WIN = 1152
NT = 8


def build_program(use_mask: bool, use_pbias: bool, reps: int = 1,
                  ps_bufs: int = 1, pb_bufs: int = 2, putil_bufs: int = 2,
                  pwin_bufs: int = 2):
    nc = bacc.Bacc("TRN2", target_bir_lowering=False, debug=False, num_devices=8)

    def din(name, shape, dt=BF16):
        return nc.dram_tensor(name, shape, dt, kind="ExternalInput").ap()

    hsT = din("hsT", [128, NCHUNK, 1024])
    wq = din("wq", [128, NCHUNK, 192])
    wk = din("wk", [128, NCHUNK, 192])
    wv = din("wv", [128, NCHUNK, 192])
    ert = din("ert", [128, 2048])
    et = din("et", [128, 2048])
    ssw = din("ssw", [64, 5, 3, 64])
    struct = din("struct", [NT, 128, 5, 1024])
    absb = din("absb", [1, 16], F32)
    idb = din("idb", [128, 128])
    if use_mask:
        maskv = din("maskv", [1, 1024])
        onesv = din("onesv", [1, 128])
    if use_pbias:
        bqv = din("bqv", [1, 192])
        bkv = din("bkv", [1, 192])
        bvv = din("bvv", [1, 192])
        onesL = din("onesL", [1, 1024])
    out = nc.dram_tensor("out", [NT, 128, 192], F32, kind="ExternalOutput").ap()

    with tile.TileContext(nc) as tc:
        for _rep in range(reps):
          with ExitStack() as ctx:
            # ---------------- constants ----------------
            consts = ctx.enter_context(tc.tile_pool(name="consts", bufs=1))

            def cload(ap_in, shape=None, dt=BF16, name=None):
                t = consts.tile(shape, dt, name=name)
                nc.sync.dma_start(t, ap_in)
                return t

            sb_hsT = cload(hsT, name="hsT", shape=[128, NCHUNK, 1024])
            sb_wk = cload(wk, name="wk", shape=[128, NCHUNK, 192])
            sb_wq = cload(wq, name="wq", shape=[128, NCHUNK, 192])
            sb_et = cload(et, name="et", shape=[128, 2048])
            sb_ert = cload(ert, name="ert", shape=[128, 2048])
            sb_ssw = cload(ssw, name="ssw", shape=[64, 5, 3, 64])
            sb_wv = cload(wv, name="wv", shape=[128, NCHUNK, 192])
            sb_absb = cload(
                bass.AP(tensor=absb.tensor, offset=0, ap=[[0, 128], [1, 16]]),
                name="absb_sb", shape=[128, 16], dt=F32,
            )
            sb_idb = cload(idb, name="idb", shape=[128, 128])
            if use_mask:
                sb_mask = cload(maskv, name="maskv", shape=[1, 1024])
                sb_ones = cload(onesv, name="onesv", shape=[1, 128])
            if use_pbias:
                sb_bq = cload(bqv, name="bqv", shape=[1, 192])
                sb_bk = cload(bkv, name="bkv", shape=[1, 192])
                sb_bv = cload(bvv, name="bvv", shape=[1, 192])
                sb_onesL = cload(onesL, name="onesL", shape=[1, 1024])

            # struct tiles: fully resident; each loaded once, on demand
            sstp = ctx.enter_context(tc.tile_pool(name="sstp", bufs=1))
            st_tiles = {}

            def load_st(t, stream=False):
                if t not in st_tiles:
                    st = sstp.tile([128, 5, 1024], BF16, tag=f"st{t}", name=f"st{t}")
                    nc.scalar.dma_start(st, struct[t])
                    st_tiles[t] = st
                return st_tiles[t]

            qkv = ctx.enter_context(tc.tile_pool(name="qkv", bufs=1))
            qd = [qkv.tile([128, 1024], BF16, tag=f"qd{h}", name=f"qd{h}") for h in range(3)]
            kd = [qkv.tile([128, 1024], BF16, tag=f"kd{h}", name=f"kd{h}") for h in range(3)]
            vsb = qkv.tile([128, NT, 192], BF16, name="vsb")

            # ---------------- projections (q/k emitted before win; v after) --
            # projection PSUM shares the putil pool (defined below)
            ptmp = ctx.enter_context(tc.tile_pool(name="ptmp", bufs=1))
            vta = ptmp.tile([128, 1024], BF16, tag="vta", name="vta")
            vtb = ptmp.tile([64, 1024], BF16, tag="vtb", name="vtb")

            def proj_mms(w_sb, bias_sb, mlo, msz, n):
                ps = pwin.tile([128, 512], F32, tag="w", name="ps")
                for c in range(NCHUNK):
                    last = (c == NCHUNK - 1) and not use_pbias
                    nc.tensor.matmul(
                        ps[0:msz, :],
                        lhsT=w_sb[:, c, ds(mlo, msz)],
                        rhs=sb_hsT[:, c, ds(512 * n, 512)],
                        start=(c == 0), stop=last,
                    )
                if use_pbias:
                    nc.tensor.matmul(
                        ps[0:msz, :],
                        lhsT=bias_sb[0:1, ds(mlo, msz)],
                        rhs=sb_onesL[0:1, ds(512 * n, 512)],
                        start=False, stop=True,
                    )
                return ps

            def proj_qk():
                for w_sb, bias_sb, dup in (
                    (sb_wk, (sb_bk if use_pbias else None), kd),
                    (sb_wq, (sb_bq if use_pbias else None), qd),
                ):
                    for n in range(2):
                        sl = ds(512 * n, 512)
                        ps = proj_mms(w_sb, bias_sb, 0, 128, n)
                        nc.scalar.activation(dup[0][0:64, sl], ps[0:64, :], AF.Copy)
                        nc.scalar.activation(dup[1][64:128, sl], ps[64:128, :], AF.Copy)
                        ps = proj_mms(w_sb, bias_sb, 128, 64, n)
                        nc.scalar.activation(dup[2][0:64, sl], ps[0:64, :], AF.Copy)
                    for h, (src, dst) in enumerate(((0, 64), (64, 0), (0, 64))):
                        nc.sync.dma_start(
                            dup[h][dst:dst + 64, :], dup[h][src:src + 64, :]
                        )

            def proj_v():
                for n in range(2):
                    sl = ds(512 * n, 512)
                    ps = proj_mms(sb_wv, (sb_bv if use_pbias else None), 0, 128, n)
                    nc.scalar.activation(vta[:, sl], ps, AF.Copy)
                    ps = proj_mms(sb_wv, (sb_bv if use_pbias else None), 128, 64, n)
                    nc.scalar.activation(vtb[:, sl], ps[0:64, :], AF.Copy)
                for t in range(NT):
                    pst = pwin.tile([128, 1024], BF16, tag="w", name="pst")
                    nc.tensor.matmul(
                        pst[:, 0:128], lhsT=vta[:, ds(128 * t, 128)], rhs=sb_idb,
                        is_transpose=True, start=True, stop=False,
                    )
                    nc.tensor.matmul(
                        pst[:, 128:192], lhsT=vtb[:, ds(128 * t, 128)],
                        rhs=sb_idb[0:64, 0:64],
                        is_transpose=True, start=False, stop=True,
                    )
                    nc.scalar.activation(vsb[:, t, 0:192], pst[:, 0:192], AF.Copy)

            dpool = ctx.enter_context(tc.tile_pool(name="dscr", bufs=1, space="DRAM"))

            # Unified PSUM pools: psS = 2-bank score tiles, putil = 1-bank
            # utility tiles (windowed rel chunks, bias, transposes, qw chunks,
            # ctx).  8 banks total.
            psS = ctx.enter_context(tc.tile_pool(name="psS", bufs=ps_bufs, space="PSUM"))
            putil = ctx.enter_context(tc.tile_pool(name="putil", bufs=putil_bufs, space="PSUM"))
            if pb_bufs:
                pbp = ctx.enter_context(tc.tile_pool(name="pbp", bufs=pb_bufs, space="PSUM"))
            else:
                pbp = putil
            pwin = ctx.enter_context(tc.tile_pool(name="pwin", bufs=pwin_bufs, space="PSUM"))
            rg = ctx.enter_context(tc.tile_pool(name="rg", bufs=2))
            wev = ctx.enter_context(tc.tile_pool(name="wev", bufs=2))
            prp = ctx.enter_context(tc.tile_pool(name="prp", bufs=2))
            prob = ctx.enter_context(tc.tile_pool(name="prob", bufs=2))
            misc = ctx.enter_context(tc.tile_pool(name="misc", bufs=2))
            ptsp = ctx.enter_context(tc.tile_pool(name="ptsp", bufs=1))
            qwp = ctx.enter_context(tc.tile_pool(name="qwp", bufs=2))

            dramQs = [dpool.tile([NT, 128, WIN], BF16, tag=f"dq{h}", name=f"dq{h}")
                      for h in range(3)]
            dramKs = [dpool.tile([NT, 128, WIN], BF16, tag=f"dk{h}", name=f"dk{h}")
                      for h in range(3)]

            qw_sbs = {}

            def qw_build(h):
                qT = qd[h]
                qw_sbs[h] = qw_sb = qwp.tile([64, 5, 1024], BF16, tag="qw", name="qw_sb")
                for i in range(5):
                    for n2 in range(2):
                        pq = pwin.tile([64, 512], F32, tag="w", name="pq")
                        nc.tensor.matmul(
                            pq,
                            lhsT=sb_ssw[:, i, h, :],
                            rhs=qT[0:64, ds(512 * n2, 512)],
                            start=True, stop=True,
                        )
                        nc.scalar.activation(qw_sb[0:64, i, ds(512 * n2, 512)], pq, AF.Copy)

            def win_unit(h, k_side, t0):
                # one (side, tile-pair) unit of the windowed rel phase:
                # 6 matmuls + 6 evacs + 1 scratch-write DMA
                qT, kT = qd[h], kd[h]
                wsrc, rhs_tab, dst, alt = (
                    (kT, sb_et, dramKs[h], 1) if k_side
                    else (qT, sb_ert, dramQs[h], 0))
                ev2 = wev.tile([128, 2, WIN], BF16, tag="wev", name="ev2")
                for pi, tt in ((0, t0), (1, t0 + 1)):
                    win = 896 - 128 * tt
                    for ci, (c0, w) in enumerate(((0, 512), (512, 512), (1024, 128))):
                        pw = pwin.tile([128, 512], F32, tag="w", name="pw")
                        nc.tensor.matmul(
                            pw[:, 0:w],
                            lhsT=wsrc[64 * pi:64 * pi + 64, ds(128 * tt, 128)],
                            rhs=rhs_tab[64 * pi:64 * pi + 64, ds(win + c0, w)],
                            start=True, stop=True, tile_position=(64 * pi, 0),
                        )
                        if (h == 0 and (ci + pi + alt) % 2 == 0) or (h > 0 and ci == 2):
                            nc.vector.tensor_copy(ev2[:, pi, ds(c0, w)], pw[:, 0:w])
                        else:
                            nc.scalar.activation(ev2[:, pi, ds(c0, w)], pw[:, 0:w], AF.Copy)
                nc.sync.dma_start(
                    bass.AP(
                        tensor=dst.tensor,
                        offset=dst.offset + t0 * 128 * WIN,
                        ap=[[WIN, 128], [128 * WIN, 2], [1, WIN]],
                    ),
                    ev2,
                )

            def win_qw(h):
                for k_side in (True, False):
                    for t0 in range(0, NT, 2):
                        win_unit(h, k_side, t0)

            state = {}

            def stageA_dma(h, t0):
                # struct first (stageB consumer), then paired diag readbacks
                load_st(t0); load_st(t0 + 1)
                dramQ, dramK = dramQs[h], dramKs[h]
                r2 = rg.tile([128, 2, 1024], BF16, tag="r2", name="r2")
                nc.sync.dma_start(
                    r2,
                    bass.AP(
                        tensor=dramQ.tensor,
                        offset=dramQ.offset + t0 * 128 * WIN + 127,
                        ap=[[WIN - 1, 128], [128 * WIN, 2], [1, 1024]],
                    ),
                )
                g2 = rg.tile([128, NT, 256], BF16, tag="g2", name="g2")
                nc.sync.dma_start(
                    g2,
                    bass.AP(
                        tensor=dramK.tensor,
                        offset=dramK.offset + 128 * t0 + 127,
                        ap=[[WIN - 1, 128], [128 * WIN, NT], [1, 256]],
                    ),
                )
                state[(h, t0)] = {"r2": r2, "g2": g2}
                state[(h, t0 + 1)] = state[(h, t0)]

            proj_qk()
            # emission order tuned for earliest possible B4(h0) start:
            # qw(h0) + first struct tiles before win(h0); h0 prefetches right
            # after win(h0).  win(h1)/win(h2) units are interleaved into the
            # previous head's round loop so the scheduler's priority order
            # alternates window work with main-loop work.
            qw_build(0)
            load_st(0); load_st(1)
            win_qw(0)
            stageA_dma(0, 0)
            stageA_dma(0, 2)
            qw_build(1)
            proj_v()

            # products: i0-i3 DVE stt (PSUM-read); i4 Act-evac + in-place
            # Pool multiply (GPSIMD has no PSUM access); 0+1 merged on Pool
            for h in range(3):
                if h == 1:
                    qw_build(2)  # qwp slot freed by B4(h0) completion
                qT, kT = qd[h], kd[h]
                dramQ, dramK, qw_sb = dramQs[h], dramKs[h], qw_sbs[h]

                def stageA(t):
                    d = state[(h, t)]
                    ii = t % 2
                    g2 = d["g2"]
                    pt2 = putil.tile([128, 1024], BF16, tag="u", name="pt2")
                    for j in range(NT):
                        nc.tensor.matmul(
                            pt2[:, ds(128 * j, 128)],
                            lhsT=g2[:, j, ds(128 * ii, 128)], rhs=sb_idb,
                            is_transpose=True,
                            start=(j == 0), stop=(j == NT - 1),
                        )
                    psc = psS.tile([128, 1024], F32, tag="s", name="psc")
                    nc.tensor.matmul(
                        psc[:, 0:512],
                        lhsT=qT[0:64, ds(128 * t, 128)], rhs=kT[0:64, 0:512],
                        start=True, stop=False, tile_position=(0, 0),
                    )
                    nc.tensor.matmul(
                        psc[:, 512:1024],
                        lhsT=qT[64:128, ds(128 * t, 128)],
                        rhs=kT[64:128, 512:1024],
                        start=True, stop=False, tile_position=(64, 0),
                    )
                    if use_mask:
                        for half in range(2):
                            sl = ds(512 * half, 512)
                            nc.tensor.matmul(
                                psc[:, sl], lhsT=sb_ones[0:1, :],
                                rhs=sb_mask[0:1, sl], start=False, stop=False,
                            )
                    d[("pt2", t)] = pt2
                    d[("psc", t)] = psc

                def biasmm(t, i, half):
                    sl = ds(512 * half, 512)
                    pb = pbp.tile([128, 512], F32, tag="u" if pbp is putil else "pb", name="pb")
                    nc.tensor.matmul(
                        pb,
                        lhsT=qw_sb[0:64, i, ds(128 * t, 128)],
                        rhs=kT[0:64, sl],
                        start=True, stop=True, tile_position=(0, 0),
                    )
                    return pb

                def stageB(t):
                    d = state[(h, t)]
                    st = st_tiles[t]
                    sums = {}
                    for i in range(5):
                        for half in range(2):
                            pb = biasmm(t, i, half)
                            pr = prp.tile([128, 512], BF16, tag=f"pd{i}{half}", name="pr")
                            if i >= 4:
                                # GPSIMD cannot read PSUM: Act evacs (+absb)
                                # into pr, Pool multiplies in place in SBUF
                                nc.scalar.activation(
                                    pr, pb, AF.Identity,
                                    bias=sb_absb[:, ds(3 * i + h, 1)])
                                nc.gpsimd.tensor_tensor(
                                    pr, pr, st[:, i, ds(512 * half, 512)], OP.mult)
                            else:
                                nc.vector.scalar_tensor_tensor(
                                    pr, in0=pb,
                                    scalar=sb_absb[:, ds(3 * i + h, 1)],
                                    in1=st[:, i, ds(512 * half, 512)],
                                    op0=OP.add, op1=OP.mult,
                                )
                            sums[(i, half)] = pr
                    # merge products 0+1 per half on Pool (SBUF-only engine)
                    for half in range(2):
                        sp = prp.tile([128, 512], BF16, tag=f"sp{half}", name="sp", bufs=2)
                        nc.gpsimd.tensor_tensor(
                            sp, sums.pop((0, half)), sums.pop((1, half)), OP.add)
                        sums[("sp", half)] = sp
                        p2, p3 = sums.pop((2, half)), sums.pop((3, half))
                        nc.vector.tensor_tensor(p2, p2, p3, OP.add)
                        sums[("sq", half)] = p2
                    d[("sums", t)] = sums

                def stageC(t):
                    d = state[(h, t)]
                    ii = t % 2
                    # rel sum: DVE add of (q-side skewed rows) + (bf16 PSUM
                    # transpose of k-side) -- 2x mode, no Act evac needed
                    rt = prp.tile([128, 1024], BF16, tag="rt", name="rt", bufs=2)
                    nc.vector.tensor_tensor(
                        rt, d[("pt2", t)], d["r2"][:, ii, :], OP.add)
                    psc, sums = d[("psc", t)], d[("sums", t)]
                    for half in range(2):
                        sl = ds(512 * half, 512)
                        joins = [rt[:, sl]]
                        joins += [v for (kk, hh), v in sums.items() if hh == half]
                        for ji, j in enumerate(joins):
                            nc.tensor.matmul(
                                psc[:, sl], lhsT=sb_idb, rhs=j,
                                start=False,
                                stop=(half == 1 and ji == len(joins) - 1),
                            )
                    probs = prob.tile([128, 1024], BF16, tag="p", name="probs")
                    rsum0 = misc.tile([128, 1], F32, tag="rs0", name="rsum0")
                    rsum1 = misc.tile([128, 1], F32, tag="rs1", name="rsum1")
                    nc.scalar.activation(probs[:, 0:512], psc[:, 0:512],
                                         AF.Exp, accum_out=rsum0)
                    nc.scalar.activation(probs[:, 512:1024], psc[:, 512:1024],
                                         AF.Exp, accum_out=rsum1)
                    rsum = misc.tile([128, 1], F32, tag="rs", name="rsum")
                    nc.vector.tensor_tensor(rsum, rsum0, rsum1, OP.add)
                    d[("probs", t)] = probs
                    d[("rsum", t)] = rsum

                def stageD(t):
                    d = state[(h, t)]
                    probs, rsum = d[("probs", t)], d[("rsum", t)]
                    ptps = putil.tile([128, 1024], BF16, tag="u", name="ptps")
                    for j in range(NT):
                        nc.tensor.matmul(
                            ptps[:, ds(128 * j, 128)],
                            lhsT=probs[:, ds(128 * j, 128)], rhs=sb_idb,
                            is_transpose=True,
                            start=(j == 0), stop=(j == NT - 1),
                        )
                    ptsb = ptsp.tile([128, 1024], BF16, tag="ptsb", name="ptsb")
                    nc.scalar.activation(ptsb, ptps, AF.Copy)
                    ctxps = putil.tile([128, 64], F32, tag="u", name="ctxps")
                    for j in range(NT):
                        nc.tensor.matmul(
                            ctxps,
                            lhsT=ptsb[:, ds(128 * j, 128)],
                            rhs=vsb[:, j, ds(64 * h, 64)],
                            start=(j == 0), stop=(j == NT - 1),
                        )
                    rec = misc.tile([128, 1], F32, tag="rc", name="rec")
                    nc.vector.reciprocal(rec, rsum)
                    cn = misc.tile([128, 64], F32, tag="cn", name="cn")
                    nc.vector.tensor_scalar_mul(cn, ctxps, rec)
                    nc.scalar.dma_start(out[t, :, ds(64 * h, 64)], cn)

                # pipelined emission; prefetch DMAs were all emitted after
                # this head's win phase (ahead of later win scratch writes in
                # DMA priority); rg pool depth flow-controls them.
                # stageB (bias+products) runs two rounds ahead of the joins --
                # it depends only on qw/kT/struct, so the C-stage never waits
                # on the vector engines
                if h != 0:
                    stageA_dma(h, 0)
                    stageA_dma(h, 2)
                stageB(0)
                for k in range(NT + 2):
                    if k % 2 == 0 and k + 4 < NT:
                        stageA_dma(h, k + 4)
                    if k + 1 < NT:
                        stageB(k + 1)
                    if 0 <= k - 2:
                        stageD(k - 2)
                    if k < NT:
                        stageA(k)
                    if 0 <= k - 1 < NT:
                        stageC(k - 1)
                    # next head's window phase, one unit per round (k-side
                    # first: the next head's g2 gather needs all of dramK)
                    if h < 2 and k < NT:
                        win_unit(h + 1, k < 4, 2 * (k % 4))

    nc.compile()
    return nc, out


_PROGRAM_CACHE = {}


def kernel(**inputs):
    hs = np.asarray(inputs["hidden_states"], np.float32)
    mask = np.asarray(inputs["attention_mask"], np.float32)
    struct = np.asarray(inputs["struct_matrix"], np.float32)
    Wq = np.asarray(inputs["Wq"], np.float32)
    bq = np.asarray(inputs["bq"], np.float32)
    Wk = np.asarray(inputs["Wk"], np.float32)
    bk = np.asarray(inputs["bk"], np.float32)
    Wv = np.asarray(inputs["Wv"], np.float32)
    bv = np.asarray(inputs["bv"], np.float32)
    E = np.asarray(inputs["dist_emb"], np.float32)
    ssw = np.asarray(inputs["ssan_w"], np.float32)
    absb = np.asarray(inputs["abs_bias"], np.float32)

    bf = ml_dtypes.bfloat16
    use_mask = bool(np.any(mask))
    use_pbias = bool(np.any(bq) or np.any(bk) or np.any(bv))

    key = (use_mask, use_pbias)
    if key not in _PROGRAM_CACHE:
        _PROGRAM_CACHE[key] = build_program(use_mask, use_pbias)
    nc, _ = _PROGRAM_CACHE[key]

    Epad = np.concatenate([E, np.zeros((1, DH), np.float32)])
    Erev = np.concatenate([E[::-1], np.zeros((1, DH), np.float32)])
    ert_half = np.ascontiguousarray(Erev.T)
    et_half = np.ascontiguousarray(Epad.T) / 8.0
    ert_np = np.concatenate([ert_half, ert_half], 0).astype(bf)
    et_np = np.concatenate([et_half, et_half], 0).astype(bf)
    idb_np = np.eye(128, dtype=np.float32).astype(bf)

    in_maps = []
    for c in range(8):
        b = c // 4
        h0 = 3 * (c % 4)
        hsT = hs[b].T
        m = {
            "hsT": np.ascontiguousarray(
                hsT.reshape(NCHUNK, 128, 1024).transpose(1, 0, 2)
            ).astype(bf),
            "wq": np.ascontiguousarray(
                (Wq[:, h0 * 64:(h0 + 3) * 64] / 8.0)
                .reshape(NCHUNK, 128, 192).transpose(1, 0, 2)
            ).astype(bf),
            "wk": np.ascontiguousarray(
                Wk[:, h0 * 64:(h0 + 3) * 64]
                .reshape(NCHUNK, 128, 192).transpose(1, 0, 2)
            ).astype(bf),
            "wv": np.ascontiguousarray(
                Wv[:, h0 * 64:(h0 + 3) * 64]
                .reshape(NCHUNK, 128, 192).transpose(1, 0, 2)
            ).astype(bf),
            "ert": ert_np,
            "et": et_np,
            "ssw": np.ascontiguousarray(
                (ssw[:, h0:h0 + 3] * 8.0).transpose(2, 0, 1, 3)
            ).astype(bf),
            "struct": np.ascontiguousarray(
                struct[:, b, 0].reshape(5, NT, 128, 1024).transpose(1, 2, 0, 3)
            ).astype(bf),
            "absb": np.concatenate(
                [absb[:, h0:h0 + 3].reshape(1, 15),
                 np.zeros((1, 1), np.float32)], 1
            ),
            "idb": idb_np,
        }
        if use_mask:
            m["maskv"] = mask[b, 0, 0].reshape(1, 1024).astype(bf)
            m["onesv"] = np.ones((1, 128), np.float32).astype(bf)
        if use_pbias:
            m["bqv"] = (bq[h0 * 64:(h0 + 3) * 64] / 8.0).reshape(1, 192).astype(bf)
            m["bkv"] = bk[h0 * 64:(h0 + 3) * 64].reshape(1, 192).astype(bf)
            m["bvv"] = bv[h0 * 64:(h0 + 3) * 64].reshape(1, 192).astype(bf)
            m["onesL"] = np.ones((1, 1024), np.float32).astype(bf)
        in_maps.append(m)

    res = run_bass_kernel_spmd(nc, in_maps, core_ids=list(range(8)))
    outs = [r["out"] for r in res.results]

    full = np.zeros((B, L, D), np.float32)
    for c in range(8):
        b = c // 4
        h0 = 3 * (c % 4)
        o = np.asarray(outs[c], np.float32).reshape(L, 192)
        for j in range(3):
            full[b, :, (h0 + j) * 64:(h0 + j + 1) * 64] = o[:, j * 64:(j + 1) * 64]
    return full
